# revision 14
# baseline (speedup 1.0000x reference)
"""MoE feed-forward kernel for Trainium2 (8 NeuronCores, SPMD expert-parallel).

Strategy
--------
Host side (inside kernel()):
  * Compute the MoE gate (softmax + top-2 + renormalize) in float64.
  * Experts are PAIRED (heaviest with lightest token count); pair p is
    handled by cores 2p and 2p+1, each taking one F-half of BOTH experts
    in the pair. Each core processes all tokens of both experts on its
    F-half; the two cores' partial down-projections sum on the host.
    This cuts token padding: capacities (CA, CB) = rounded max of the
    larger/smaller member over pairs, instead of global max over experts.
  * Shared expert is sharded 2D: token-quarter (c % 4) x F-half (c // 4).
  * Weights/activations are packed to bf16 in matmul-native layouts.
Device side (one Bass/Tile program, run on all 8 cores with different data):
  * Phase order: shared up/gate -> routed up/gate -> routed down ->
    shared down. The small xs/sug tensors unblock the tensor engine
    ~13us after launch while the big xe/wug/wd DMAs stream behind.
  * up/gate:  uT[f,:] = sum_k wug[k,f].T @ xT[k,:]   (F on partitions)
    - token columns split into equal chunks (<=512) so every matmul's
      stream time covers its LDWEIGHTS.
  * a = silu(u) * rw * g  (routing weight folded in as a per-column scale)
  * down (flipped): yT[hb,:] = sum_f wd[f,hb].T @ aT[f,:]
    - h-blocks on partitions, tokens as the moving free dim: no token-row
      padding to 128, LDWEIGHTS hidden under the >=248-col stream.
  * Outputs are transposed ([H, tokens]); host transposes + scatter-adds.
"""

import os
import numpy as np
import ml_dtypes

import concourse.bacc as bacc
import concourse.mybir as mybir
import concourse.tile as tile
from concourse.bass_utils import run_bass_kernel_spmd

BF16 = mybir.dt.bfloat16
F32 = mybir.dt.float32
P = 128

# Problem dims (hardcoded per contest rules; kernel.py must be self-contained).
H = 2048
F = 5632
E = 8
TOP_K = 2
T = 2048
N_CORES = 8

LAST_EXEC_NS = None
LAST_RESULTS = None

_compiled = {}


def _equal_chunks(total, maxn=512, base_off=0):
    """Split `total` columns into equal-ish chunks <= maxn, multiples of 4."""
    n = (total + maxn - 1) // maxn
    base = (total // n) & ~3
    out, s = [], 0
    for i in range(n - 1):
        out.append((base_off + s, base))
        s += base
    out.append((base_off + s, total - s))
    return out


def _build(CA, CB, *, h=H, st=None):
    """Build + compile the SPMD program: two routed segments (expert pair,
    F-half each) with capacities CA and CB, plus the shared expert."""
    kt = h // P
    fth = (F // P) // 2     # routed F-half tiles per core
    fs = (F // P) // 2      # shared F-half tiles per core
    st = st if st is not None else T // 4
    hb_n = h // P
    Ctot = CA + CB

    nc = bacc.Bacc(
        "TRN2",
        target_bir_lowering=False,
        debug=False,
        enable_asserts=False,
        num_devices=N_CORES,
    )

    xe_d = nc.dram_tensor("xe", [P, kt, Ctot], BF16, kind="ExternalInput")
    xs_d = nc.dram_tensor("xs", [P, kt, st], BF16, kind="ExternalInput")
    rwb_d = nc.dram_tensor("rwb", [P, Ctot], F32, kind="ExternalInput")
    wug_d = nc.dram_tensor("wug", [2, P, fth, 2, kt, P], BF16, kind="ExternalInput")
    wd_d = nc.dram_tensor("wd", [2, hb_n, P, fth, P], BF16, kind="ExternalInput")
    sug_d = nc.dram_tensor("sug", [P, fs, 2, kt, P], BF16, kind="ExternalInput")
    sd_d = nc.dram_tensor("sd", [hb_n, P, fs, P], BF16, kind="ExternalInput")
    ye_d = nc.dram_tensor("ye", [h, Ctot], F32, kind="ExternalOutput")
    ys_d = nc.dram_tensor("ys", [h, st], F32, kind="ExternalOutput")

    # (c0, cw, segment) chunk lists; segment selects the expert weight set.
    r_chunks = [(c0, cw, 0) for (c0, cw) in _equal_chunks(CA)] + \
               [(c0, cw, 1) for (c0, cw) in _equal_chunks(CB, base_off=CA)]
    s_chunks = [(c0, cw, 0) for (c0, cw) in _equal_chunks(st)]

    with tile.TileContext(nc) as tc:
        with (
            tc.tile_pool(name="const", bufs=1) as cpool,
            tc.tile_pool(name="acts", bufs=1) as apool,
            tc.tile_pool(name="wug_s", bufs=4) as wpool,
            tc.tile_pool(name="wd_s", bufs=4) as wdpool,
            tc.tile_pool(name="tmp", bufs=2) as tpool,
            tc.tile_pool(name="osb", bufs=3) as opool,
            tc.tile_pool(name="ps_u", bufs=2, space="PSUM") as pu_pool,
            tc.tile_pool(name="ps_g", bufs=2, space="PSUM") as pg_pool,
            tc.tile_pool(name="ps_y", bufs=2, space="PSUM") as py_pool,
        ):
            # DMA order is FIFO across the packet engines, so enqueue in
            # the order compute needs it: xs k-group 0, then the first
            # shared weight tile, then the rest of xs. The big xe/rwb
            # prefetch is spread across the early shared f-tiles (via
            # after_fi) so it never starves the sug stream.
            xs_sb = cpool.tile([P, kt, st], BF16, tag="xs", name="xs_sb")
            kg = min(4, kt)
            # First matmul needs only xs k-tile 0 and the u-half of the
            # first shared weight tile; enqueue exactly that first.
            nc.sync.dma_start(xs_sb[:, 0:1], xs_d[:, 0:1])
            w_sug0 = wpool.tile([P, 2, kt, P], BF16, tag="wug", name="w_as2_0_0")
            nc.sync.dma_start(w_sug0[:, 0], sug_d[:, 0, 0])
            nc.sync.dma_start(xs_sb[:, 1:2], xs_d[:, 1:2])
            nc.sync.dma_start(w_sug0[:, 1], sug_d[:, 0, 1])
            nc.sync.dma_start(xs_sb[:, 2:4], xs_d[:, 2:4])
            for k0 in range(kg, kt, kg):
                nc.sync.dma_start(xs_sb[:, k0:k0 + kg], xs_d[:, k0:k0 + kg])
            xe_sb = cpool.tile([P, kt, Ctot], BF16, tag="xe", name="xe_sb")
            rwb_sb = cpool.tile([P, Ctot], F32, tag="rwb", name="rwb_sb")

            def prefetch_routed(fi):
                # One xe k-group per early shared f-tile, then rwb.
                if 1 <= fi <= kt // kg:
                    k0 = (fi - 1) * kg
                    nc.sync.dma_start(xe_sb[:, k0:k0 + kg], xe_d[:, k0:k0 + kg])
                elif fi == kt // kg + 1:
                    nc.sync.dma_start(rwb_sb[:], rwb_d[:])

            def up_gate(n_ft, w_drams, x_sb, chunk_list, ctot, out_tag, rwb,
                        after_fi=None, w0=None):
                # One 3-D activation tile for the whole phase (few sems).
                a_all = apool.tile([P, n_ft, ctot], BF16, tag=out_tag,
                                   name=f"a_{out_tag}")
                n_seg = len(w_drams)
                for fi in range(n_ft):
                    ws = []
                    for s in range(n_seg):
                        if fi == 0 and s == 0 and w0 is not None:
                            ws.append(w0)
                            continue
                        w = wpool.tile([P, 2, kt, P], BF16, tag="wug",
                                       name=f"w_{out_tag}_{fi}_{s}")
                        nc.sync.dma_start(w[:], w_drams[s][:, fi])
                        ws.append(w)
                    if after_fi is not None:
                        after_fi(fi)
                    for (c0, cw, s) in chunk_list:
                        w = ws[s]
                        pu = pu_pool.tile([P, cw], F32, tag="pu", name=f"pu_{out_tag}_{fi}_{c0}")
                        pg = pg_pool.tile([P, cw], F32, tag="pg", name=f"pg_{out_tag}_{fi}_{c0}")
                        for k in range(kt):
                            nc.tensor.matmul(pu[:], w[:, 0, k], x_sb[:, k, c0:c0 + cw],
                                             start=(k == 0), stop=(k == kt - 1))
                        for k in range(kt):
                            nc.tensor.matmul(pg[:], w[:, 1, k], x_sb[:, k, c0:c0 + cw],
                                             start=(k == 0), stop=(k == kt - 1))
                        su = tpool.tile([P, cw], F32, tag="su", name=f"su_{out_tag}_{fi}_{c0}")
                        nc.scalar.activation(su[:], pu[:], mybir.ActivationFunctionType.Sigmoid)
                        nc.vector.tensor_mul(su[:], su[:], pu[:])
                        if rwb is not None:
                            nc.vector.tensor_mul(su[:], su[:], rwb[:, c0:c0 + cw])
                        nc.vector.tensor_mul(a_all[:, fi, c0:c0 + cw], su[:], pg[:])
                return a_all

            def down_T(n_ft, a_all, w_drams, out_dram, chunk_list,
                       last_hb_chunks=None):
                n_seg = len(w_drams)
                for hb in range(hb_n):
                    gts = []
                    for s in range(n_seg):
                        gt = wdpool.tile([P, n_ft, P], BF16, tag="wd",
                                         name=f"wd_{out_dram.name}_{hb}_{s}")
                        nc.sync.dma_start(gt[:], w_drams[s][hb])
                        gts.append(gt)
                    cl = chunk_list
                    if last_hb_chunks is not None and hb == hb_n - 1:
                        cl = last_hb_chunks
                    for (c0, cw, s) in cl:
                        py = py_pool.tile([P, cw], F32, tag="py",
                                          name=f"py_{out_dram.name}_{hb}_{c0}")
                        for j in range(n_ft):
                            nc.tensor.matmul(py[:], gts[s][:, j], a_all[:, j, c0:c0 + cw],
                                             start=(j == 0), stop=(j == n_ft - 1))
                        o = opool.tile([P, cw], F32, tag="o",
                                       name=f"o_{out_dram.name}_{hb}_{c0}")
                        nc.vector.tensor_copy(o[:], py[:])
                        nc.sync.dma_start(out_dram[hb * P:(hb + 1) * P, c0:c0 + cw], o[:])

            as2 = up_gate(fs, [sug_d], xs_sb, s_chunks, st, "as2", None,
                          after_fi=prefetch_routed, w0=w_sug0)
            aT = up_gate(fth, [wug_d[0], wug_d[1]], xe_sb, r_chunks, Ctot,
                         "aT", rwb_sb)
            down_T(fth, aT, [wd_d[0], wd_d[1]], ye_d, r_chunks)
            # Shrink chunks toward the end of the final h-block so the
            # kernel's tail (last PSUM drain + copy + output DMA) is short.
            half = st // 2
            s_last = [(0, half, 0), (half, half // 2, 0),
                      (half + half // 2, half - half // 2, 0)]
            down_T(fs, as2, [sd_d], ys_d, s_chunks, last_hb_chunks=s_last)

    nc.compile()
    return nc


def _pack_ug(wu, wg):
    """[H, Fp] x2 (f32) -> [P, ft, 2, kt, P] bf16."""
    kt = wu.shape[0] // P
    ft = wu.shape[1] // P
    ru = wu.reshape(kt, P, ft, P).transpose(1, 2, 0, 3)
    rg = wg.reshape(kt, P, ft, P).transpose(1, 2, 0, 3)
    return np.ascontiguousarray(
        np.stack([ru, rg], axis=2)).astype(ml_dtypes.bfloat16)


def _pack_down_T(wd):
    """[Fp, H] f32 -> [hb, P, ft, P] bf16 (h-block major, f on partitions)."""
    fp, h = wd.shape
    ft = fp // P
    hb = h // P
    r = wd.reshape(ft, P, hb, P).transpose(2, 1, 0, 3)
    return np.ascontiguousarray(r).astype(ml_dtypes.bfloat16)


def _pack_xT(xrows):
    """[n, H] f32 -> [P, kt, n] bf16."""
    n, h = xrows.shape
    kt = h // P
    return np.ascontiguousarray(
        xrows.reshape(n, kt, P).transpose(2, 1, 0)).astype(ml_dtypes.bfloat16)


def _try_install_ntff_shim():
    """Register the NTFF profile hook that this container's antenv lacks,
    so run_bass_kernel_spmd(trace=True) can capture HW exec time."""
    try:
        import sys
        import types

        if "antenv.axon_hooks" not in sys.modules:
            import trn_agent_boot.trn_boot as tb

            hook = tb._ntff_profile_via_ctypes("/opt/axon/libaxon_pjrt.so")
            if hook is None:
                return False
            mod = types.ModuleType("antenv.axon_hooks")
            mod.get_axon_ntff_profile_hook = lambda: hook
            mod.set_axon_ntff_profile_hook = lambda h: None
            sys.modules["antenv.axon_hooks"] = mod
        import concourse.bass_utils as bu

        bu.upload_artifacts = lambda tmpdir: f"file://{tmpdir}"
        return True
    except Exception as e:  # pragma: no cover - profiling is best-effort
        print("ntff shim unavailable:", e)
        return False


def kernel(hidden_state, gate_w, w_gate, w_up, w_down, sw_gate, sw_up, sw_down):
    global LAST_EXEC_NS, LAST_RESULTS

    x = np.asarray(hidden_state, dtype=np.float32).reshape(-1, H)
    gate_w = np.asarray(gate_w, dtype=np.float32)
    w_gate = np.asarray(w_gate, dtype=np.float32)
    w_up = np.asarray(w_up, dtype=np.float32)
    w_down = np.asarray(w_down, dtype=np.float32)
    sw_gate = np.asarray(sw_gate, dtype=np.float32)
    sw_up = np.asarray(sw_up, dtype=np.float32)
    sw_down = np.asarray(sw_down, dtype=np.float32)

    # ---- gate (float64 on host; decisions match the f32 reference far
    # inside the observed 2e-5 top-k score gap) ----
    logits = x.astype(np.float64) @ gate_w.T.astype(np.float64)
    logits -= logits.max(axis=-1, keepdims=True)
    ex = np.exp(logits)
    score = ex / ex.sum(axis=-1, keepdims=True)
    top2 = np.argsort(-score, axis=-1, kind="stable")[:, :TOP_K]
    tw = np.take_along_axis(score, top2, axis=-1)
    tw = tw / (tw.sum(axis=-1, keepdims=True) + 1e-20)

    idx_e, w_e = [], []
    for e in range(E):
        sel = top2 == e
        rows = np.flatnonzero(sel.any(axis=1))
        ww = (tw * sel)[rows].sum(axis=1)
        idx_e.append(rows)
        w_e.append(ww.astype(np.float32))
    counts = np.array([len(i) for i in idx_e])

    # Pair heaviest with lightest so the uniform capacities (CA, CB) are
    # as tight as possible across the 4 pairs.
    order = np.argsort(-counts, kind="stable")
    pairs = [(int(order[p]), int(order[7 - p])) for p in range(4)]
    CA = max(int(np.ceil(max(counts[a] for a, _ in pairs) / 4)) * 4, 64)
    CB = max(int(np.ceil(max(counts[b] for _, b in pairs) / 4)) * 4, 64)
    Ctot = CA + CB

    if (CA, CB) not in _compiled:
        _compiled[(CA, CB)] = _build(CA, CB)
    nc = _compiled[(CA, CB)]

    st = T // 4
    fs = (F // P) // 2
    fw = fs * P  # F-half width

    in_maps = []
    sug_cache = {}
    sd_cache = {}
    xe_cache = {}
    rwb_cache = {}
    for c in range(N_CORES):
        q = c % 4
        fh = c // 4
        if fh not in sug_cache:
            cols = slice(fh * fw, (fh + 1) * fw)
            sug_cache[fh] = _pack_ug(sw_up[0][:, cols], sw_gate[0][:, cols])
            sd_cache[fh] = _pack_down_T(sw_down[0][cols, :])
        p = c // 2
        hr = c % 2
        ea, eb = pairs[p]
        if p not in xe_cache:
            xe = np.zeros((Ctot, H), np.float32)
            xe[:counts[ea]] = x[idx_e[ea]]
            xe[CA:CA + counts[eb]] = x[idx_e[eb]]
            xe_cache[p] = _pack_xT(xe)
            rw = np.zeros(Ctot, np.float32)
            rw[:counts[ea]] = w_e[ea]
            rw[CA:CA + counts[eb]] = w_e[eb]
            rwb_cache[p] = np.ascontiguousarray(np.broadcast_to(rw, (P, Ctot)))
        rcols = slice(hr * fw, (hr + 1) * fw)
        in_maps.append({
            "xe": xe_cache[p],
            "xs": _pack_xT(x[q * st:(q + 1) * st]),
            "rwb": rwb_cache[p],
            "wug": np.stack([
                _pack_ug(w_up[ea][:, rcols], w_gate[ea][:, rcols]),
                _pack_ug(w_up[eb][:, rcols], w_gate[eb][:, rcols])]),
            "wd": np.stack([
                _pack_down_T(w_down[ea][rcols, :]),
                _pack_down_T(w_down[eb][rcols, :])]),
            "sug": sug_cache[fh],
            "sd": sd_cache[fh],
        })

    trace = bool(int(os.environ.get("KERNEL_TRACE", "0")))
    if trace:
        trace = _try_install_ntff_shim()
    tmpdir = os.environ.get("KERNEL_TRACE_DIR") or None
    res = run_bass_kernel_spmd(
        nc, in_maps, list(range(N_CORES)), trace=trace, tmpdir=tmpdir)
    LAST_EXEC_NS = res.exec_time_ns
    LAST_RESULTS = res

    y = np.zeros((T, H), np.float32)
    for c in range(N_CORES):
        p = c // 2
        ea, eb = pairs[p]
        ye = res.results[c]["ye"]
        y[idx_e[ea]] += ye[:, :counts[ea]].T
        y[idx_e[eb]] += ye[:, CA:CA + counts[eb]].T
    for c in range(N_CORES):
        q = c % 4
        y[q * st:(q + 1) * st] += res.results[c]["ys"].T

    return y.reshape(2, 1024, H)


# revision 15
# speedup vs baseline: 1.0007x; 1.0007x over previous
"""MoE feed-forward kernel for Trainium2 (8 NeuronCores, SPMD expert-parallel).

Strategy
--------
Host side (inside kernel()):
  * Compute the MoE gate (softmax + top-2 + renormalize) in float64.
  * Experts are PAIRED (heaviest with lightest token count); pair p is
    handled by cores 2p and 2p+1, each taking one F-half of BOTH experts
    in the pair. Each core processes all tokens of both experts on its
    F-half; the two cores' partial down-projections sum on the host.
    This cuts token padding: capacities (CA, CB) = rounded max of the
    larger/smaller member over pairs, instead of global max over experts.
  * Shared expert is sharded 2D: token-quarter (c % 4) x F-half (c // 4).
  * Weights/activations are packed to bf16 in matmul-native layouts.
Device side (one Bass/Tile program, run on all 8 cores with different data):
  * Phase order: shared up/gate -> routed up/gate -> routed down ->
    shared down. The small xs/sug tensors unblock the tensor engine
    ~13us after launch while the big xe/wug/wd DMAs stream behind.
  * up/gate:  uT[f,:] = sum_k wug[k,f].T @ xT[k,:]   (F on partitions)
    - token columns split into equal chunks (<=512) so every matmul's
      stream time covers its LDWEIGHTS.
  * a = silu(u) * rw * g  (routing weight folded in as a per-column scale)
  * down (flipped): yT[hb,:] = sum_f wd[f,hb].T @ aT[f,:]
    - h-blocks on partitions, tokens as the moving free dim: no token-row
      padding to 128, LDWEIGHTS hidden under the >=248-col stream.
  * Outputs are transposed ([H, tokens]); host transposes + scatter-adds.
"""

import os
import numpy as np
import ml_dtypes

import concourse.bacc as bacc
import concourse.mybir as mybir
import concourse.tile as tile
from concourse.bass_utils import run_bass_kernel_spmd

BF16 = mybir.dt.bfloat16
F32 = mybir.dt.float32
P = 128

# Problem dims (hardcoded per contest rules; kernel.py must be self-contained).
H = 2048
F = 5632
E = 8
TOP_K = 2
T = 2048
N_CORES = 8

LAST_EXEC_NS = None
LAST_RESULTS = None

_compiled = {}


def _equal_chunks(total, maxn=512, base_off=0):
    """Split `total` columns into equal-ish chunks <= maxn, multiples of 4."""
    n = (total + maxn - 1) // maxn
    base = (total // n) & ~3
    out, s = [], 0
    for i in range(n - 1):
        out.append((base_off + s, base))
        s += base
    out.append((base_off + s, total - s))
    return out


def _build(CA, CB, *, h=H, st=None):
    """Build + compile the SPMD program: two routed segments (expert pair,
    F-half each) with capacities CA and CB, plus the shared expert."""
    kt = h // P
    fth = (F // P) // 2     # routed F-half tiles per core
    fs = (F // P) // 2      # shared F-half tiles per core
    st = st if st is not None else T // 4
    hb_n = h // P
    Ctot = CA + CB

    nc = bacc.Bacc(
        "TRN2",
        target_bir_lowering=False,
        debug=False,
        enable_asserts=False,
        num_devices=N_CORES,
    )

    xe_d = nc.dram_tensor("xe", [P, kt, Ctot], BF16, kind="ExternalInput")
    xs_d = nc.dram_tensor("xs", [P, kt, st], BF16, kind="ExternalInput")
    rwb_d = nc.dram_tensor("rwb", [P, Ctot], F32, kind="ExternalInput")
    wug_d = nc.dram_tensor("wug", [2, P, fth, 2, kt, P], BF16, kind="ExternalInput")
    wd_d = nc.dram_tensor("wd", [2, hb_n, P, fth, P], BF16, kind="ExternalInput")
    sug_d = nc.dram_tensor("sug", [P, fs, 2, kt, P], BF16, kind="ExternalInput")
    sd_d = nc.dram_tensor("sd", [hb_n, P, fs, P], BF16, kind="ExternalInput")
    ye_d = nc.dram_tensor("ye", [h, Ctot], F32, kind="ExternalOutput")
    ys_d = nc.dram_tensor("ys", [h, st], F32, kind="ExternalOutput")

    # (c0, cw, segment) chunk lists; segment selects the expert weight set.
    r_chunks = [(c0, cw, 0) for (c0, cw) in _equal_chunks(CA)] + \
               [(c0, cw, 1) for (c0, cw) in _equal_chunks(CB, base_off=CA)]
    s_chunks = [(c0, cw, 0) for (c0, cw) in _equal_chunks(st)]

    with tile.TileContext(nc) as tc:
        with (
            tc.tile_pool(name="const", bufs=1) as cpool,
            tc.tile_pool(name="acts", bufs=1) as apool,
            tc.tile_pool(name="wug_s", bufs=4) as wpool,
            tc.tile_pool(name="wd_s", bufs=4) as wdpool,
            tc.tile_pool(name="tmp", bufs=2) as tpool,
            tc.tile_pool(name="osb", bufs=3) as opool,
            tc.tile_pool(name="ps_u", bufs=2, space="PSUM") as pu_pool,
            tc.tile_pool(name="ps_g", bufs=2, space="PSUM") as pg_pool,
            tc.tile_pool(name="ps_y", bufs=2, space="PSUM") as py_pool,
        ):
            # DMA order is FIFO across the packet engines, so enqueue in
            # the order compute needs it: xs k-group 0, then the first
            # shared weight tile, then the rest of xs. The big xe/rwb
            # prefetch is spread across the early shared f-tiles (via
            # after_fi) so it never starves the sug stream.
            xs_sb = cpool.tile([P, kt, st], BF16, tag="xs", name="xs_sb")
            kg = min(4, kt)
            nc.sync.dma_start(xs_sb[:, 0:1], xs_d[:, 0:1])
            w_sug0 = wpool.tile([P, 2, kt, P], BF16, tag="wug", name="w_as2_0_0")
            nc.sync.dma_start(w_sug0[:], sug_d[:, 0])
            nc.sync.dma_start(xs_sb[:, 1:2], xs_d[:, 1:2])
            nc.sync.dma_start(xs_sb[:, 2:4], xs_d[:, 2:4])
            for k0 in range(kg, kt, kg):
                nc.sync.dma_start(xs_sb[:, k0:k0 + kg], xs_d[:, k0:k0 + kg])
            xe_sb = cpool.tile([P, kt, Ctot], BF16, tag="xe", name="xe_sb")
            rwb_sb = cpool.tile([P, Ctot], F32, tag="rwb", name="rwb_sb")

            def prefetch_routed(fi):
                # One xe k-group per early shared f-tile, then rwb.
                if 1 <= fi <= kt // kg:
                    k0 = (fi - 1) * kg
                    nc.sync.dma_start(xe_sb[:, k0:k0 + kg], xe_d[:, k0:k0 + kg])
                elif fi == kt // kg + 1:
                    nc.sync.dma_start(rwb_sb[:], rwb_d[:])

            def up_gate(n_ft, w_drams, x_sb, chunk_list, ctot, out_tag, rwb,
                        after_fi=None, w0=None):
                # One 3-D activation tile for the whole phase (few sems).
                a_all = apool.tile([P, n_ft, ctot], BF16, tag=out_tag,
                                   name=f"a_{out_tag}")
                n_seg = len(w_drams)
                for fi in range(n_ft):
                    ws = []
                    for s in range(n_seg):
                        if fi == 0 and s == 0 and w0 is not None:
                            ws.append(w0)
                            continue
                        w = wpool.tile([P, 2, kt, P], BF16, tag="wug",
                                       name=f"w_{out_tag}_{fi}_{s}")
                        nc.sync.dma_start(w[:], w_drams[s][:, fi])
                        ws.append(w)
                    if after_fi is not None:
                        after_fi(fi)
                    for (c0, cw, s) in chunk_list:
                        w = ws[s]
                        pu = pu_pool.tile([P, cw], F32, tag="pu", name=f"pu_{out_tag}_{fi}_{c0}")
                        pg = pg_pool.tile([P, cw], F32, tag="pg", name=f"pg_{out_tag}_{fi}_{c0}")
                        for k in range(kt):
                            nc.tensor.matmul(pu[:], w[:, 0, k], x_sb[:, k, c0:c0 + cw],
                                             start=(k == 0), stop=(k == kt - 1))
                        for k in range(kt):
                            nc.tensor.matmul(pg[:], w[:, 1, k], x_sb[:, k, c0:c0 + cw],
                                             start=(k == 0), stop=(k == kt - 1))
                        su = tpool.tile([P, cw], F32, tag="su", name=f"su_{out_tag}_{fi}_{c0}")
                        nc.scalar.activation(su[:], pu[:], mybir.ActivationFunctionType.Sigmoid)
                        nc.vector.tensor_mul(su[:], su[:], pu[:])
                        if rwb is not None:
                            nc.vector.tensor_mul(su[:], su[:], rwb[:, c0:c0 + cw])
                        nc.vector.tensor_mul(a_all[:, fi, c0:c0 + cw], su[:], pg[:])
                return a_all

            def down_T(n_ft, a_all, w_drams, out_dram, chunk_list,
                       last_hb_chunks=None):
                n_seg = len(w_drams)
                for hb in range(hb_n):
                    gts = []
                    for s in range(n_seg):
                        gt = wdpool.tile([P, n_ft, P], BF16, tag="wd",
                                         name=f"wd_{out_dram.name}_{hb}_{s}")
                        nc.sync.dma_start(gt[:], w_drams[s][hb])
                        gts.append(gt)
                    cl = chunk_list
                    if last_hb_chunks is not None and hb == hb_n - 1:
                        cl = last_hb_chunks
                    for (c0, cw, s) in cl:
                        py = py_pool.tile([P, cw], F32, tag="py",
                                          name=f"py_{out_dram.name}_{hb}_{c0}")
                        for j in range(n_ft):
                            nc.tensor.matmul(py[:], gts[s][:, j], a_all[:, j, c0:c0 + cw],
                                             start=(j == 0), stop=(j == n_ft - 1))
                        o = opool.tile([P, cw], F32, tag="o",
                                       name=f"o_{out_dram.name}_{hb}_{c0}")
                        nc.vector.tensor_copy(o[:], py[:])
                        nc.sync.dma_start(out_dram[hb * P:(hb + 1) * P, c0:c0 + cw], o[:])

            as2 = up_gate(fs, [sug_d], xs_sb, s_chunks, st, "as2", None,
                          after_fi=prefetch_routed, w0=w_sug0)
            aT = up_gate(fth, [wug_d[0], wug_d[1]], xe_sb, r_chunks, Ctot,
                         "aT", rwb_sb)
            down_T(fth, aT, [wd_d[0], wd_d[1]], ye_d, r_chunks)
            # Shrink chunks toward the end of the final h-block so the
            # kernel's tail (last PSUM drain + copy + output DMA) is short.
            half = st // 2
            s_last = [(0, half, 0), (half, half // 2, 0),
                      (half + half // 2, half - half // 2, 0)]
            down_T(fs, as2, [sd_d], ys_d, s_chunks, last_hb_chunks=s_last)

    nc.compile()
    return nc


def _pack_ug(wu, wg):
    """[H, Fp] x2 (f32) -> [P, ft, 2, kt, P] bf16."""
    kt = wu.shape[0] // P
    ft = wu.shape[1] // P
    ru = wu.reshape(kt, P, ft, P).transpose(1, 2, 0, 3)
    rg = wg.reshape(kt, P, ft, P).transpose(1, 2, 0, 3)
    return np.ascontiguousarray(
        np.stack([ru, rg], axis=2)).astype(ml_dtypes.bfloat16)


def _pack_down_T(wd):
    """[Fp, H] f32 -> [hb, P, ft, P] bf16 (h-block major, f on partitions)."""
    fp, h = wd.shape
    ft = fp // P
    hb = h // P
    r = wd.reshape(ft, P, hb, P).transpose(2, 1, 0, 3)
    return np.ascontiguousarray(r).astype(ml_dtypes.bfloat16)


def _pack_xT(xrows):
    """[n, H] f32 -> [P, kt, n] bf16."""
    n, h = xrows.shape
    kt = h // P
    return np.ascontiguousarray(
        xrows.reshape(n, kt, P).transpose(2, 1, 0)).astype(ml_dtypes.bfloat16)


def _try_install_ntff_shim():
    """Register the NTFF profile hook that this container's antenv lacks,
    so run_bass_kernel_spmd(trace=True) can capture HW exec time."""
    try:
        import sys
        import types

        if "antenv.axon_hooks" not in sys.modules:
            import trn_agent_boot.trn_boot as tb

            hook = tb._ntff_profile_via_ctypes("/opt/axon/libaxon_pjrt.so")
            if hook is None:
                return False
            mod = types.ModuleType("antenv.axon_hooks")
            mod.get_axon_ntff_profile_hook = lambda: hook
            mod.set_axon_ntff_profile_hook = lambda h: None
            sys.modules["antenv.axon_hooks"] = mod
        import concourse.bass_utils as bu

        bu.upload_artifacts = lambda tmpdir: f"file://{tmpdir}"
        return True
    except Exception as e:  # pragma: no cover - profiling is best-effort
        print("ntff shim unavailable:", e)
        return False


def kernel(hidden_state, gate_w, w_gate, w_up, w_down, sw_gate, sw_up, sw_down):
    global LAST_EXEC_NS, LAST_RESULTS

    x = np.asarray(hidden_state, dtype=np.float32).reshape(-1, H)
    gate_w = np.asarray(gate_w, dtype=np.float32)
    w_gate = np.asarray(w_gate, dtype=np.float32)
    w_up = np.asarray(w_up, dtype=np.float32)
    w_down = np.asarray(w_down, dtype=np.float32)
    sw_gate = np.asarray(sw_gate, dtype=np.float32)
    sw_up = np.asarray(sw_up, dtype=np.float32)
    sw_down = np.asarray(sw_down, dtype=np.float32)

    # ---- gate (float64 on host; decisions match the f32 reference far
    # inside the observed 2e-5 top-k score gap) ----
    logits = x.astype(np.float64) @ gate_w.T.astype(np.float64)
    logits -= logits.max(axis=-1, keepdims=True)
    ex = np.exp(logits)
    score = ex / ex.sum(axis=-1, keepdims=True)
    top2 = np.argsort(-score, axis=-1, kind="stable")[:, :TOP_K]
    tw = np.take_along_axis(score, top2, axis=-1)
    tw = tw / (tw.sum(axis=-1, keepdims=True) + 1e-20)

    idx_e, w_e = [], []
    for e in range(E):
        sel = top2 == e
        rows = np.flatnonzero(sel.any(axis=1))
        ww = (tw * sel)[rows].sum(axis=1)
        idx_e.append(rows)
        w_e.append(ww.astype(np.float32))
    counts = np.array([len(i) for i in idx_e])

    # Pair heaviest with lightest so the uniform capacities (CA, CB) are
    # as tight as possible across the 4 pairs.
    order = np.argsort(-counts, kind="stable")
    pairs = [(int(order[p]), int(order[7 - p])) for p in range(4)]
    CA = max(int(np.ceil(max(counts[a] for a, _ in pairs) / 4)) * 4, 64)
    CB = max(int(np.ceil(max(counts[b] for _, b in pairs) / 4)) * 4, 64)
    Ctot = CA + CB

    if (CA, CB) not in _compiled:
        _compiled[(CA, CB)] = _build(CA, CB)
    nc = _compiled[(CA, CB)]

    st = T // 4
    fs = (F // P) // 2
    fw = fs * P  # F-half width

    in_maps = []
    sug_cache = {}
    sd_cache = {}
    xe_cache = {}
    rwb_cache = {}
    for c in range(N_CORES):
        q = c % 4
        fh = c // 4
        if fh not in sug_cache:
            cols = slice(fh * fw, (fh + 1) * fw)
            sug_cache[fh] = _pack_ug(sw_up[0][:, cols], sw_gate[0][:, cols])
            sd_cache[fh] = _pack_down_T(sw_down[0][cols, :])
        p = c // 2
        hr = c % 2
        ea, eb = pairs[p]
        if p not in xe_cache:
            xe = np.zeros((Ctot, H), np.float32)
            xe[:counts[ea]] = x[idx_e[ea]]
            xe[CA:CA + counts[eb]] = x[idx_e[eb]]
            xe_cache[p] = _pack_xT(xe)
            rw = np.zeros(Ctot, np.float32)
            rw[:counts[ea]] = w_e[ea]
            rw[CA:CA + counts[eb]] = w_e[eb]
            rwb_cache[p] = np.ascontiguousarray(np.broadcast_to(rw, (P, Ctot)))
        rcols = slice(hr * fw, (hr + 1) * fw)
        in_maps.append({
            "xe": xe_cache[p],
            "xs": _pack_xT(x[q * st:(q + 1) * st]),
            "rwb": rwb_cache[p],
            "wug": np.stack([
                _pack_ug(w_up[ea][:, rcols], w_gate[ea][:, rcols]),
                _pack_ug(w_up[eb][:, rcols], w_gate[eb][:, rcols])]),
            "wd": np.stack([
                _pack_down_T(w_down[ea][rcols, :]),
                _pack_down_T(w_down[eb][rcols, :])]),
            "sug": sug_cache[fh],
            "sd": sd_cache[fh],
        })

    trace = bool(int(os.environ.get("KERNEL_TRACE", "0")))
    if trace:
        trace = _try_install_ntff_shim()
    tmpdir = os.environ.get("KERNEL_TRACE_DIR") or None
    res = run_bass_kernel_spmd(
        nc, in_maps, list(range(N_CORES)), trace=trace, tmpdir=tmpdir)
    LAST_EXEC_NS = res.exec_time_ns
    LAST_RESULTS = res

    y = np.zeros((T, H), np.float32)
    for c in range(N_CORES):
        p = c // 2
        ea, eb = pairs[p]
        ye = res.results[c]["ye"]
        y[idx_e[ea]] += ye[:, :counts[ea]].T
        y[idx_e[eb]] += ye[:, CA:CA + counts[eb]].T
    for c in range(N_CORES):
        q = c % 4
        y[q * st:(q + 1) * st] += res.results[c]["ys"].T

    return y.reshape(2, 1024, H)


# revision 17
# speedup vs baseline: 1.0018x; 1.0010x over previous
"""MoE feed-forward kernel for Trainium2 (8 NeuronCores, SPMD expert-parallel).

Strategy
--------
Host side (inside kernel()):
  * Compute the MoE gate (softmax + top-2 + renormalize) in float64.
  * Experts are PAIRED (heaviest with lightest token count); pair p is
    handled by cores 2p and 2p+1, each taking one F-half of BOTH experts
    in the pair. Each core processes all tokens of both experts on its
    F-half; the two cores' partial down-projections sum on the host.
    This cuts token padding: capacities (CA, CB) = rounded max of the
    larger/smaller member over pairs, instead of global max over experts.
  * Shared expert is sharded 2D: token-quarter (c % 4) x F-half (c // 4).
  * Weights/activations are packed to bf16 in matmul-native layouts.
Device side (one Bass/Tile program, run on all 8 cores with different data):
  * Phase order: shared up/gate -> routed up/gate -> routed down ->
    shared down. The small xs/sug tensors unblock the tensor engine
    ~13us after launch while the big xe/wug/wd DMAs stream behind.
  * up/gate:  uT[f,:] = sum_k wug[k,f].T @ xT[k,:]   (F on partitions)
    - token columns split into equal chunks (<=512) so every matmul's
      stream time covers its LDWEIGHTS.
  * a = silu(u) * rw * g  (routing weight folded in as a per-column scale)
  * down (flipped): yT[hb,:] = sum_f wd[f,hb].T @ aT[f,:]
    - h-blocks on partitions, tokens as the moving free dim: no token-row
      padding to 128, LDWEIGHTS hidden under the >=248-col stream.
  * Outputs are transposed ([H, tokens]); host transposes + scatter-adds.
"""

import os
import numpy as np
import ml_dtypes

import concourse.bacc as bacc
import concourse.mybir as mybir
import concourse.tile as tile
from concourse.bass_utils import run_bass_kernel_spmd

BF16 = mybir.dt.bfloat16
F32 = mybir.dt.float32
P = 128

# Problem dims (hardcoded per contest rules; kernel.py must be self-contained).
H = 2048
F = 5632
E = 8
TOP_K = 2
T = 2048
N_CORES = 8

LAST_EXEC_NS = None
LAST_RESULTS = None

_compiled = {}


def _equal_chunks(total, maxn=512, base_off=0):
    """Split `total` columns into equal-ish chunks <= maxn, multiples of 2."""
    n = (total + maxn - 1) // maxn
    base = (total // n) & ~1
    out, s = [], 0
    for i in range(n - 1):
        out.append((base_off + s, base))
        s += base
    out.append((base_off + s, total - s))
    return out


def _build(CA, CB, *, h=H, st=None):
    """Build + compile the SPMD program: two routed segments (expert pair,
    F-half each) with capacities CA and CB, plus the shared expert."""
    kt = h // P
    fth = (F // P) // 2     # routed F-half tiles per core
    fs = (F // P) // 2      # shared F-half tiles per core
    st = st if st is not None else T // 4
    hb_n = h // P
    Ctot = CA + CB

    nc = bacc.Bacc(
        "TRN2",
        target_bir_lowering=False,
        debug=False,
        enable_asserts=False,
        num_devices=N_CORES,
    )

    xe_d = nc.dram_tensor("xe", [P, kt, Ctot], BF16, kind="ExternalInput")
    xs_d = nc.dram_tensor("xs", [P, kt, st], BF16, kind="ExternalInput")
    rwb_d = nc.dram_tensor("rwb", [P, Ctot], F32, kind="ExternalInput")
    wug_d = nc.dram_tensor("wug", [2, P, fth, 2, kt, P], BF16, kind="ExternalInput")
    wd_d = nc.dram_tensor("wd", [2, hb_n, P, fth, P], BF16, kind="ExternalInput")
    sug_d = nc.dram_tensor("sug", [P, fs, 2, kt, P], BF16, kind="ExternalInput")
    sd_d = nc.dram_tensor("sd", [hb_n, P, fs, P], BF16, kind="ExternalInput")
    ye_d = nc.dram_tensor("ye", [h, Ctot], F32, kind="ExternalOutput")
    ys_d = nc.dram_tensor("ys", [h, st], F32, kind="ExternalOutput")

    # (c0, cw, segment) chunk lists; segment selects the expert weight set.
    r_chunks = [(c0, cw, 0) for (c0, cw) in _equal_chunks(CA)] + \
               [(c0, cw, 1) for (c0, cw) in _equal_chunks(CB, base_off=CA)]
    s_chunks = [(c0, cw, 0) for (c0, cw) in _equal_chunks(st)]

    with tile.TileContext(nc) as tc:
        with (
            tc.tile_pool(name="const", bufs=1) as cpool,
            tc.tile_pool(name="acts", bufs=1) as apool,
            tc.tile_pool(name="wug_s", bufs=4) as wpool,
            tc.tile_pool(name="wd_s", bufs=4) as wdpool,
            tc.tile_pool(name="tmp", bufs=2) as tpool,
            tc.tile_pool(name="osb", bufs=3) as opool,
            tc.tile_pool(name="ps_u", bufs=2, space="PSUM") as pu_pool,
            tc.tile_pool(name="ps_g", bufs=2, space="PSUM") as pg_pool,
            tc.tile_pool(name="ps_y", bufs=2, space="PSUM") as py_pool,
        ):
            # DMA order is FIFO across the packet engines, so enqueue in
            # the order compute needs it: xs k-group 0, then the first
            # shared weight tile, then the rest of xs. The big xe/rwb
            # prefetch is spread across the early shared f-tiles (via
            # after_fi) so it never starves the sug stream.
            xs_sb = cpool.tile([P, kt, st], BF16, tag="xs", name="xs_sb")
            kg = min(4, kt)
            nc.sync.dma_start(xs_sb[:, 0:1], xs_d[:, 0:1])
            w_sug0 = wpool.tile([P, 2, kt, P], BF16, tag="wug", name="w_as2_0_0")
            nc.sync.dma_start(w_sug0[:], sug_d[:, 0])
            nc.sync.dma_start(xs_sb[:, 1:2], xs_d[:, 1:2])
            nc.sync.dma_start(xs_sb[:, 2:4], xs_d[:, 2:4])
            for k0 in range(kg, kt, kg):
                nc.sync.dma_start(xs_sb[:, k0:k0 + kg], xs_d[:, k0:k0 + kg])
            xe_sb = cpool.tile([P, kt, Ctot], BF16, tag="xe", name="xe_sb")
            rwb_sb = cpool.tile([P, Ctot], F32, tag="rwb", name="rwb_sb")

            def prefetch_routed(fi):
                # One xe k-group per early shared f-tile, then rwb.
                if 1 <= fi <= kt // kg:
                    k0 = (fi - 1) * kg
                    nc.sync.dma_start(xe_sb[:, k0:k0 + kg], xe_d[:, k0:k0 + kg])
                elif fi == kt // kg + 1:
                    nc.sync.dma_start(rwb_sb[:], rwb_d[:])

            def up_gate(n_ft, w_drams, x_sb, chunk_list, ctot, out_tag, rwb,
                        after_fi=None, w0=None):
                # One 3-D activation tile for the whole phase (few sems).
                a_all = apool.tile([P, n_ft, ctot], BF16, tag=out_tag,
                                   name=f"a_{out_tag}")
                n_seg = len(w_drams)
                for fi in range(n_ft):
                    ws = []
                    for s in range(n_seg):
                        if fi == 0 and s == 0 and w0 is not None:
                            ws.append(w0)
                            continue
                        w = wpool.tile([P, 2, kt, P], BF16, tag="wug",
                                       name=f"w_{out_tag}_{fi}_{s}")
                        nc.sync.dma_start(w[:], w_drams[s][:, fi])
                        ws.append(w)
                    if after_fi is not None:
                        after_fi(fi)
                    for (c0, cw, s) in chunk_list:
                        w = ws[s]
                        pu = pu_pool.tile([P, cw], F32, tag="pu", name=f"pu_{out_tag}_{fi}_{c0}")
                        pg = pg_pool.tile([P, cw], F32, tag="pg", name=f"pg_{out_tag}_{fi}_{c0}")
                        for k in range(kt):
                            nc.tensor.matmul(pu[:], w[:, 0, k], x_sb[:, k, c0:c0 + cw],
                                             start=(k == 0), stop=(k == kt - 1))
                        for k in range(kt):
                            nc.tensor.matmul(pg[:], w[:, 1, k], x_sb[:, k, c0:c0 + cw],
                                             start=(k == 0), stop=(k == kt - 1))
                        su = tpool.tile([P, cw], F32, tag="su", name=f"su_{out_tag}_{fi}_{c0}")
                        nc.scalar.activation(su[:], pu[:], mybir.ActivationFunctionType.Sigmoid)
                        nc.vector.tensor_mul(su[:], su[:], pu[:])
                        if rwb is not None:
                            nc.vector.tensor_mul(su[:], su[:], rwb[:, c0:c0 + cw])
                        nc.vector.tensor_mul(a_all[:, fi, c0:c0 + cw], su[:], pg[:])
                return a_all

            def down_T(n_ft, a_all, w_drams, out_dram, chunk_list,
                       last_hb_chunks=None):
                n_seg = len(w_drams)
                for hb in range(hb_n):
                    gts = []
                    for s in range(n_seg):
                        gt = wdpool.tile([P, n_ft, P], BF16, tag="wd",
                                         name=f"wd_{out_dram.name}_{hb}_{s}")
                        nc.sync.dma_start(gt[:], w_drams[s][hb])
                        gts.append(gt)
                    cl = chunk_list
                    if last_hb_chunks is not None and hb == hb_n - 1:
                        cl = last_hb_chunks
                    for (c0, cw, s) in cl:
                        py = py_pool.tile([P, cw], F32, tag="py",
                                          name=f"py_{out_dram.name}_{hb}_{c0}")
                        for j in range(n_ft):
                            nc.tensor.matmul(py[:], gts[s][:, j], a_all[:, j, c0:c0 + cw],
                                             start=(j == 0), stop=(j == n_ft - 1))
                        o = opool.tile([P, cw], F32, tag="o",
                                       name=f"o_{out_dram.name}_{hb}_{c0}")
                        nc.vector.tensor_copy(o[:], py[:])
                        nc.sync.dma_start(out_dram[hb * P:(hb + 1) * P, c0:c0 + cw], o[:])

            as2 = up_gate(fs, [sug_d], xs_sb, s_chunks, st, "as2", None,
                          after_fi=prefetch_routed, w0=w_sug0)
            aT = up_gate(fth, [wug_d[0], wug_d[1]], xe_sb, r_chunks, Ctot,
                         "aT", rwb_sb)
            down_T(fth, aT, [wd_d[0], wd_d[1]], ye_d, r_chunks)
            # Shrink chunks toward the end of the final h-block so the
            # kernel's tail (last PSUM drain + copy + output DMA) is short.
            half = st // 2
            s_last = [(0, half, 0), (half, half // 2, 0),
                      (half + half // 2, half - half // 2, 0)]
            down_T(fs, as2, [sd_d], ys_d, s_chunks, last_hb_chunks=s_last)

    nc.compile()
    return nc


def _pack_ug(wu, wg):
    """[H, Fp] x2 (f32) -> [P, ft, 2, kt, P] bf16."""
    kt = wu.shape[0] // P
    ft = wu.shape[1] // P
    ru = wu.reshape(kt, P, ft, P).transpose(1, 2, 0, 3)
    rg = wg.reshape(kt, P, ft, P).transpose(1, 2, 0, 3)
    return np.ascontiguousarray(
        np.stack([ru, rg], axis=2)).astype(ml_dtypes.bfloat16)


def _pack_down_T(wd):
    """[Fp, H] f32 -> [hb, P, ft, P] bf16 (h-block major, f on partitions)."""
    fp, h = wd.shape
    ft = fp // P
    hb = h // P
    r = wd.reshape(ft, P, hb, P).transpose(2, 1, 0, 3)
    return np.ascontiguousarray(r).astype(ml_dtypes.bfloat16)


def _pack_xT(xrows):
    """[n, H] f32 -> [P, kt, n] bf16."""
    n, h = xrows.shape
    kt = h // P
    return np.ascontiguousarray(
        xrows.reshape(n, kt, P).transpose(2, 1, 0)).astype(ml_dtypes.bfloat16)


def _try_install_ntff_shim():
    """Register the NTFF profile hook that this container's antenv lacks,
    so run_bass_kernel_spmd(trace=True) can capture HW exec time."""
    try:
        import sys
        import types

        if "antenv.axon_hooks" not in sys.modules:
            import trn_agent_boot.trn_boot as tb

            hook = tb._ntff_profile_via_ctypes("/opt/axon/libaxon_pjrt.so")
            if hook is None:
                return False
            mod = types.ModuleType("antenv.axon_hooks")
            mod.get_axon_ntff_profile_hook = lambda: hook
            mod.set_axon_ntff_profile_hook = lambda h: None
            sys.modules["antenv.axon_hooks"] = mod
        import concourse.bass_utils as bu

        bu.upload_artifacts = lambda tmpdir: f"file://{tmpdir}"
        return True
    except Exception as e:  # pragma: no cover - profiling is best-effort
        print("ntff shim unavailable:", e)
        return False


def kernel(hidden_state, gate_w, w_gate, w_up, w_down, sw_gate, sw_up, sw_down):
    global LAST_EXEC_NS, LAST_RESULTS

    x = np.asarray(hidden_state, dtype=np.float32).reshape(-1, H)
    gate_w = np.asarray(gate_w, dtype=np.float32)
    w_gate = np.asarray(w_gate, dtype=np.float32)
    w_up = np.asarray(w_up, dtype=np.float32)
    w_down = np.asarray(w_down, dtype=np.float32)
    sw_gate = np.asarray(sw_gate, dtype=np.float32)
    sw_up = np.asarray(sw_up, dtype=np.float32)
    sw_down = np.asarray(sw_down, dtype=np.float32)

    # ---- gate (float64 on host; decisions match the f32 reference far
    # inside the observed 2e-5 top-k score gap) ----
    logits = x.astype(np.float64) @ gate_w.T.astype(np.float64)
    logits -= logits.max(axis=-1, keepdims=True)
    ex = np.exp(logits)
    score = ex / ex.sum(axis=-1, keepdims=True)
    top2 = np.argsort(-score, axis=-1, kind="stable")[:, :TOP_K]
    tw = np.take_along_axis(score, top2, axis=-1)
    tw = tw / (tw.sum(axis=-1, keepdims=True) + 1e-20)

    idx_e, w_e = [], []
    for e in range(E):
        sel = top2 == e
        rows = np.flatnonzero(sel.any(axis=1))
        ww = (tw * sel)[rows].sum(axis=1)
        idx_e.append(rows)
        w_e.append(ww.astype(np.float32))
    counts = np.array([len(i) for i in idx_e])

    # Pair heaviest with lightest so the uniform capacities (CA, CB) are
    # as tight as possible across the 4 pairs.
    order = np.argsort(-counts, kind="stable")
    pairs = [(int(order[p]), int(order[7 - p])) for p in range(4)]
    CA = max(int(np.ceil(max(counts[a] for a, _ in pairs) / 2)) * 2, 64)
    CB = max(int(np.ceil(max(counts[b] for _, b in pairs) / 2)) * 2, 64)
    Ctot = CA + CB

    if (CA, CB) not in _compiled:
        _compiled[(CA, CB)] = _build(CA, CB)
    nc = _compiled[(CA, CB)]

    st = T // 4
    fs = (F // P) // 2
    fw = fs * P  # F-half width

    in_maps = []
    sug_cache = {}
    sd_cache = {}
    xe_cache = {}
    rwb_cache = {}
    for c in range(N_CORES):
        q = c % 4
        fh = c // 4
        if fh not in sug_cache:
            cols = slice(fh * fw, (fh + 1) * fw)
            sug_cache[fh] = _pack_ug(sw_up[0][:, cols], sw_gate[0][:, cols])
            sd_cache[fh] = _pack_down_T(sw_down[0][cols, :])
        p = c // 2
        hr = c % 2
        ea, eb = pairs[p]
        if p not in xe_cache:
            xe = np.zeros((Ctot, H), np.float32)
            xe[:counts[ea]] = x[idx_e[ea]]
            xe[CA:CA + counts[eb]] = x[idx_e[eb]]
            xe_cache[p] = _pack_xT(xe)
            rw = np.zeros(Ctot, np.float32)
            rw[:counts[ea]] = w_e[ea]
            rw[CA:CA + counts[eb]] = w_e[eb]
            rwb_cache[p] = np.ascontiguousarray(np.broadcast_to(rw, (P, Ctot)))
        rcols = slice(hr * fw, (hr + 1) * fw)
        in_maps.append({
            "xe": xe_cache[p],
            "xs": _pack_xT(x[q * st:(q + 1) * st]),
            "rwb": rwb_cache[p],
            "wug": np.stack([
                _pack_ug(w_up[ea][:, rcols], w_gate[ea][:, rcols]),
                _pack_ug(w_up[eb][:, rcols], w_gate[eb][:, rcols])]),
            "wd": np.stack([
                _pack_down_T(w_down[ea][rcols, :]),
                _pack_down_T(w_down[eb][rcols, :])]),
            "sug": sug_cache[fh],
            "sd": sd_cache[fh],
        })

    trace = bool(int(os.environ.get("KERNEL_TRACE", "0")))
    if trace:
        trace = _try_install_ntff_shim()
    tmpdir = os.environ.get("KERNEL_TRACE_DIR") or None
    res = run_bass_kernel_spmd(
        nc, in_maps, list(range(N_CORES)), trace=trace, tmpdir=tmpdir)
    LAST_EXEC_NS = res.exec_time_ns
    LAST_RESULTS = res

    y = np.zeros((T, H), np.float32)
    for c in range(N_CORES):
        p = c // 2
        ea, eb = pairs[p]
        ye = res.results[c]["ye"]
        y[idx_e[ea]] += ye[:, :counts[ea]].T
        y[idx_e[eb]] += ye[:, CA:CA + counts[eb]].T
    for c in range(N_CORES):
        q = c % 4
        y[q * st:(q + 1) * st] += res.results[c]["ys"].T

    return y.reshape(2, 1024, H)
